# revision 1
# baseline (speedup 1.0000x reference)
"""Trainium2 Bass kernel for nn_NodeProcessor (GNN message passing).

Strategy (8 NeuronCores, SPMD, no collectives):
  - Host sorts edges by destination node and shards NODES (6250/core);
    each core receives exactly the edges destined to its node shard, so no
    cross-core reduction is needed.
  - On device, segment-sum is computed per 128-node tile as a sequence of
    128-edge-chunk matmuls accumulating in PSUM:
        agg_T[f, n] += sum_e edge_chunk[e, f] * S[e, n],
    where S[e, n] = (j_rel[e] == n) is a one-hot selection matrix built by
    an is_equal compare against a constant iota row.  Edges are pre-sorted,
    so each chunk belongs to one node tile and spans few nodes: chunk 0 of
    a tile writes the full 128-wide region (start=True clears the
    accumulator), later chunks compare/accumulate only a W=32-wide window
    at a host-baked column offset.  S matrices are built in batched
    compare ops (16 windows / 7 full tiles at a time).
  - MLP: h1_T = relu(W1.T @ [x_T; agg_T] + b1) feature-major; h2 is then
    produced directly in node-major form by using h1_T as the stationary
    operand (h2[n, o] = sum_h h1_T[h, n] * W2[h, o]), with b2 added via a
    rank-1 ones-column matmul — no on-device transposes anywhere.
  - LayerNorm node-major via bn_stats/bn_aggr + fused (v-mu)*rstd
    tensor_scalar; gamma scale + residual add run on GpSimd; the residual
    (x + beta) is folded on host.  Output is stored node-major.
  - Per-core tile processing order is chosen (descending chunk count) so
    one SPMD program (a common per-tile chunk schedule) fits all cores.
    Scatter for tile t+1 is emitted ahead of the MLP of tile t to keep the
    in-order PE queue fed.

Matmul inputs are bf16; accumulation is f32 in PSUM; LayerNorm statistics
and the residual add are f32.
"""

import os
import sys

import numpy as np

for _p in ("/opt/trn_rl_repo", "/root/.axon_site/_ro/trn_rl_repo"):
    if os.path.isdir(_p) and _p not in sys.path:
        sys.path.insert(0, _p)

import ml_dtypes

import concourse.bacc as bacc
import concourse.bass as bass
import concourse.tile as tile
from concourse import mybir
from concourse.bass_utils import run_bass_kernel_spmd

BF16 = ml_dtypes.bfloat16

N_NODES = 50000
N_EDGES = 600000
D = 128           # node/edge feature dim
H = 256           # hidden dim
NCORE = 8
NSHARD = N_NODES // NCORE      # 6250 real nodes per core
P = 128                        # partition / tile size
NT = 49                        # node tiles per core (49*128 = 6272 >= 6250)
G = 7                          # tile group size (NT = G*G)
NPAD = NT * P                  # padded nodes per core
L = 32                         # edge chunks per DMA load
W = 32                         # scatter window width (max common span is 27)
SB = 16                        # windows per batched S-build op
LN_EPS = 1e-5
PAD_J = 200.0                  # j_rel sentinel for padded edge rows

S_ENGINE = os.environ.get("S_ENGINE", "vector")      # S-build engine
T2_ENGINE = os.environ.get("T2_ENGINE", "gpsimd")    # gamma-mult engine
Y_ENGINE = os.environ.get("Y_ENGINE", "gpsimd")      # residual-add engine

if os.environ.get("KERNEL_LDW_OPT"):
    from concourse import bass_utils as _bu

    _orig_run_command = _bu.run_command

    def _patched_run_command(argv, **kw):
        argv = [
            "--enable-ldw-opt=true" if a == "--enable-ldw-opt=false" else a
            for a in argv
        ]
        return _orig_run_command(argv, **kw)

    _bu.run_command = _patched_run_command


def _prep_host(x, edge_index, edge_attr, W1, b1, W2, b2, ln_g, ln_b):
    """Sort/shard/pack all inputs."""
    j = np.asarray(edge_index[1], dtype=np.int64)
    perm = np.argsort(j, kind="stable")
    js = j[perm]

    edge_attr_bf = np.asarray(edge_attr, dtype=BF16)
    x = np.asarray(x, dtype=np.float32)
    ln_b = np.asarray(ln_b, dtype=np.float32)

    bounds = np.searchsorted(js, np.arange(NCORE + 1) * NSHARD)

    core_info = []
    for c in range(NCORE):
        es, ee = bounds[c], bounds[c + 1]
        jl = js[es:ee] - c * NSHARD           # local node id, 0..6249
        rows = perm[es:ee]                    # rows into edge_attr
        cnt = np.bincount(jl // P, minlength=NT)  # edges per tile
        ch = -(-cnt // P)                     # ceil chunks per tile
        tile_perm = np.argsort(-ch, kind="stable")  # descending chunk count
        core_info.append((jl, rows, cnt, ch, tile_perm))

    sorted_ch = np.stack([ci[3][ci[4]] for ci in core_info])  # [NCORE, NT]
    schedule = np.maximum(sorted_ch.max(axis=0), 1).astype(np.int64)
    nchunk = int(schedule.sum())
    nload = -(-nchunk // L)
    nc_tot = nload * L

    chunk_base = np.zeros(NT + 1, dtype=np.int64)
    np.cumsum(schedule, out=chunk_base[1:])

    # Tile-relative j_rel per chunk slot per core; chunk 0 of a tile is
    # full-width, later chunks use a common W-wide window (max over cores
    # of the span must fit — verified below).
    minj = np.full((NCORE, nc_tot), 1 << 30, dtype=np.int64)
    maxj = np.full((NCORE, nc_tot), -1, dtype=np.int64)
    per_core_fill = []
    for c in range(NCORE):
        jl, rows, cnt, ch, tile_perm = core_info[c]
        tile_start = np.zeros(NT + 1, dtype=np.int64)
        np.cumsum(cnt, out=tile_start[1:])
        ridx = np.zeros(nc_tot * P, dtype=np.int64)
        jrel_t = np.full(nc_tot * P, -1, dtype=np.int64)  # tile-relative
        for s in range(NT):
            T = int(tile_perm[s])
            n = int(cnt[T])
            dst = chunk_base[s] * P
            ridx[dst : dst + n] = rows[tile_start[T] : tile_start[T] + n]
            jrel_t[dst : dst + n] = jl[tile_start[T] : tile_start[T] + n] - T * P
        jr2 = jrel_t.reshape(nc_tot, P)
        valid = jr2 >= 0
        anyv = valid.any(axis=1)
        mn = np.where(anyv, np.where(valid, jr2, 1 << 30).min(axis=1), 1 << 30)
        mx = np.where(anyv, np.where(valid, jr2, -1).max(axis=1), -1)
        minj[c] = mn
        maxj[c] = mx
        per_core_fill.append((ridx, jrel_t))

    woff = np.clip(minj.min(axis=0), 0, P - W)
    woff[chunk_base[:-1]] = 0  # chunk 0 full width
    fw = np.zeros(nc_tot, dtype=bool)
    fw[chunk_base[:-1]] = True
    width = np.where(fw, P, W)
    assert (maxj.max(axis=0) < woff + width).all(), "chunk span exceeds window"

    in_maps = []
    for c in range(NCORE):
        jl, rows, cnt, ch, tile_perm = core_info[c]
        ridx, jrel_t = per_core_fill[c]
        jr2 = jrel_t.reshape(nc_tot, P).astype(np.float32) - woff[:, None]
        jr2[jrel_t.reshape(nc_tot, P) < 0] = PAD_J

        ea_all = edge_attr_bf[ridx]           # [nc_tot*P, D]
        ea_pack = (
            ea_all.reshape(nload, L, P, D)
            .transpose(0, 2, 1, 3)
            .reshape(nload, P, L * D)
            .copy()
        )
        jr_pack = np.ascontiguousarray(jr2.T.astype(BF16))  # [P, nc_tot]
        # chunk-0 columns (tile-relative j_rel) gathered into slot order
        jr0_pack = np.ascontiguousarray(jr2[chunk_base[:-1]].T.astype(BF16))

        # x shard: bf16 feature-major tiles (MLP input) and f32 node-major
        # residual (+ beta folded), ordered by tile_perm, G tiles per DMA.
        xs = np.zeros((NPAD, D), dtype=np.float32)
        xs[:NSHARD] = x[c * NSHARD : (c + 1) * NSHARD]
        xt = xs.reshape(NT, P, D).transpose(0, 2, 1)[tile_perm]  # [NT, f, n]
        xbf_pack = (
            xt.astype(BF16).reshape(G, G, D, P).transpose(0, 2, 1, 3)
            .reshape(G, D, G * P).copy()
        )
        xfn = (xs + ln_b[None, :]).reshape(NT, P, D)[tile_perm]  # [NT, n, f]
        xf_pack = (
            xfn.reshape(G, G, P, D).transpose(0, 2, 1, 3)
            .reshape(G, P, G * D).copy()
        )

        vecs = np.concatenate(
            [np.asarray(b1, np.float32).ravel()]
        ).reshape(H, 1)

        in_maps.append(
            {
                "ea": ea_pack,
                "jr": jr_pack,
                "jr0": jr0_pack,
                "xbf": xbf_pack,
                "xf": xf_pack,
                "W1d": np.asarray(W1, BF16),
                "W2d": np.asarray(W2, BF16),
                "vecs": vecs,
                "b2r": np.asarray(b2, BF16).reshape(1, D),
                "gb": np.tile(np.asarray(ln_g, np.float32), (P, 1)).astype(BF16),
                "iota_d": np.tile(np.arange(P, dtype=np.float32), (P, 1)).astype(BF16),
                "iotaw_d": np.tile(np.arange(W, dtype=np.float32), (P, SB)).astype(BF16),
                "iotag_d": np.tile(np.arange(P, dtype=np.float32), (P, G)).astype(BF16),
            }
        )

    meta = (schedule, woff, nload, nc_tot)
    return in_maps, meta, [ci[4] for ci in core_info]


def _build_program(meta):
    schedule, woff, nload, nc_tot = meta
    f32 = mybir.dt.float32
    bf16 = mybir.dt.bfloat16
    AF = mybir.ActivationFunctionType
    OP = mybir.AluOpType

    nc = bacc.Bacc("TRN2", target_bir_lowering=False, debug=False,
                   num_devices=NCORE)
    s_eng = getattr(nc, S_ENGINE)
    t2_eng = getattr(nc, T2_ENGINE)
    y_eng = getattr(nc, Y_ENGINE)

    ea_d = nc.dram_tensor("ea", [nload, P, L * D], bf16, kind="ExternalInput").ap()
    jr_d = nc.dram_tensor("jr", [P, nc_tot], bf16, kind="ExternalInput").ap()
    jr0_d = nc.dram_tensor("jr0", [P, NT], bf16, kind="ExternalInput").ap()
    xbf_d = nc.dram_tensor("xbf", [G, D, G * P], bf16, kind="ExternalInput").ap()
    xf_d = nc.dram_tensor("xf", [G, P, G * D], f32, kind="ExternalInput").ap()
    w1_d = nc.dram_tensor("W1d", [H, H], bf16, kind="ExternalInput").ap()
    w2_d = nc.dram_tensor("W2d", [H, D], bf16, kind="ExternalInput").ap()
    vecs_d = nc.dram_tensor("vecs", [H, 1], f32, kind="ExternalInput").ap()
    b2r_d = nc.dram_tensor("b2r", [1, D], bf16, kind="ExternalInput").ap()
    gb_d = nc.dram_tensor("gb", [P, D], bf16, kind="ExternalInput").ap()
    iota_dr = nc.dram_tensor("iota_d", [P, P], bf16, kind="ExternalInput").ap()
    iotaw_dr = nc.dram_tensor("iotaw_d", [P, SB * W], bf16, kind="ExternalInput").ap()
    iotag_dr = nc.dram_tensor("iotag_d", [P, G * P], bf16, kind="ExternalInput").ap()
    out_d = nc.dram_tensor("outN", [G, P, G * D], f32, kind="ExternalOutput").ap()

    with tile.TileContext(nc) as tc:
        with (
            tc.tile_pool(name="consts", bufs=1) as consts,
            tc.tile_pool(name="edges", bufs=6) as epool,
            tc.tile_pool(name="xg", bufs=2) as xpool,
            tc.tile_pool(name="yg", bufs=2) as ypool,
            tc.tile_pool(name="s0", bufs=3) as s0pool,
            tc.tile_pool(name="sm", bufs=8) as spool,
            tc.tile_pool(name="work", bufs=3) as wpool,
            tc.tile_pool(name="ps", bufs=1, space="PSUM") as pspool,
            tc.tile_pool(name="ps2", bufs=3, space="PSUM") as ps2pool,
            tc.tile_pool(name="psagg", bufs=3, space="PSUM") as psagg,
        ):
            # ---- constants ----
            jr_sb = consts.tile([P, nc_tot], bf16)
            nc.sync.dma_start(out=jr_sb[:], in_=jr_d[:])
            jr0_sb = consts.tile([P, NT], bf16, tag="jr0")
            nc.sync.dma_start(out=jr0_sb[:], in_=jr0_d[:])
            iota_sb = consts.tile([P, P], bf16)
            nc.sync.dma_start(out=iota_sb[:], in_=iota_dr[:])
            iotaw_sb = consts.tile([P, SB * W], bf16, tag="iotaw")
            nc.sync.dma_start(out=iotaw_sb[:], in_=iotaw_dr[:])
            iotag_sb = consts.tile([P, G * P], bf16, tag="iotag")
            nc.sync.dma_start(out=iotag_sb[:], in_=iotag_dr[:])
            gb_sb = consts.tile([P, D], bf16, tag="gb")
            nc.sync.dma_start(out=gb_sb[:], in_=gb_d[:])
            b2r_sb = consts.tile([1, D], bf16, tag="b2r")
            nc.sync.dma_start(out=b2r_sb[:], in_=b2r_d[:])
            ones_row = consts.tile([1, P], bf16, tag="ones_row")
            nc.vector.memset(ones_row[:], 1.0)

            w1xa = consts.tile([P, P], bf16, tag="w1xa")
            nc.sync.dma_start(out=w1xa[:], in_=w1_d[0:P, 0:P])
            w1xb = consts.tile([P, P], bf16, tag="w1xb")
            nc.sync.dma_start(out=w1xb[:], in_=w1_d[0:P, P : 2 * P])
            w1ga = consts.tile([P, P], bf16, tag="w1ga")
            nc.sync.dma_start(out=w1ga[:], in_=w1_d[P : 2 * P, 0:P])
            w1gb = consts.tile([P, P], bf16, tag="w1gb")
            nc.sync.dma_start(out=w1gb[:], in_=w1_d[P : 2 * P, P : 2 * P])
            w2a = consts.tile([P, P], bf16, tag="w2a")
            nc.sync.dma_start(out=w2a[:], in_=w2_d[0:P, :])
            w2b = consts.tile([P, P], bf16, tag="w2b")
            nc.sync.dma_start(out=w2b[:], in_=w2_d[P : 2 * P, :])

            b1a = consts.tile([P, 1], f32, tag="b1a")
            nc.sync.dma_start(out=b1a[:], in_=vecs_d[0:P, :])
            b1b = consts.tile([P, 1], f32, tag="b1b")
            nc.sync.dma_start(out=b1b[:], in_=vecs_d[P : 2 * P, :])
            eps_sb = consts.tile([P, 1], f32, tag="eps")
            nc.vector.memset(eps_sb[:], LN_EPS)

            def iota3(qn, width):
                a = iota_sb[:, 0:width]
                return bass.AP(tensor=a.tensor, offset=a.offset,
                               ap=[a.ap[0], [0, qn], a.ap[1]])

            load_tiles = {}

            def ensure_load(ld):
                if ld < 0 or ld >= nload or ld in load_tiles:
                    return
                et = epool.tile([P, L * D], bf16, tag="ea", name=f"ea{ld}")
                nc.sync.dma_start(out=et[:], in_=ea_d[ld])
                load_tiles[ld] = et

            def edge_slice(c):
                ld, sl = divmod(c, L)
                ensure_load(ld)
                ensure_load(ld + 1)
                ensure_load(ld + 2)
                return load_tiles[ld][:, sl * D : (sl + 1) * D]

            chunk_base = np.zeros(NT + 1, dtype=np.int64)
            np.cumsum(schedule, out=chunk_base[1:])

            # batched full-width S for the chunk-0s of one tile group
            s0_tiles = {}

            def s0_group(gi):
                if gi not in s0_tiles:
                    S0g = s0pool.tile([P, G * P], bf16, tag="S0g")
                    s_eng.tensor_tensor(
                        out=S0g[:].rearrange("p (q w) -> p q w", w=P),
                        in0=jr0_sb[:, gi * G : (gi + 1) * G].to_broadcast(
                            [P, G, P]
                        ),
                        in1=iotag_sb[:].rearrange("p (q w) -> p q w", w=P),
                        op=OP.is_equal,
                    )
                    s0_tiles[gi] = S0g
                return s0_tiles[gi]

            aggT_pairs = {}
            s_of = {}

            def sbuild_tile(t):
                """Build the selection matrices for tile t (runs well ahead
                of the matmuls so the in-order DVE queue never gates PE)."""
                gi, ti = divmod(t, G)
                c0 = int(chunk_base[t])
                ncch = int(schedule[t])
                s0_group(gi)
                sbs = []
                for q0 in range(1, ncch, SB):
                    qn = min(SB, ncch - q0)
                    Sb = spool.tile([P, SB * W], bf16, tag="Sb",
                                    name=f"Sb{t}_{q0}")
                    s_eng.tensor_tensor(
                        out=Sb[:, : qn * W].rearrange("p (q w) -> p q w", w=W),
                        in0=jr_sb[:, c0 + q0 : c0 + q0 + qn].to_broadcast(
                            [P, qn, W]
                        ),
                        in1=iotaw_sb[:, : qn * W].rearrange(
                            "p (q w) -> p q w", w=W
                        ),
                        op=OP.is_equal,
                    )
                    sbs.append(Sb)
                s_of[t] = sbs

            def scatter_tile(t):
                gi, ti = divmod(t, G)
                c0 = int(chunk_base[t])
                ncch = int(schedule[t])
                agg_ps = psagg.tile([P, P], f32, tag="agg")
                S0g = s0_group(gi)
                nc.tensor.matmul(
                    agg_ps[:], lhsT=edge_slice(c0),
                    rhs=S0g[:, ti * P : (ti + 1) * P],
                    start=True, stop=(ncch == 1),
                )
                sbs = s_of.pop(t)
                for bi, q0 in enumerate(range(1, ncch, SB)):
                    qn = min(SB, ncch - q0)
                    Sb = sbs[bi]
                    for i in range(qn):
                        c = c0 + q0 + i
                        w = int(woff[c])
                        nc.tensor.matmul(
                            agg_ps[:, w : w + W],
                            lhsT=edge_slice(c),
                            rhs=Sb[:, i * W : (i + 1) * W],
                            start=False,
                            stop=(c == c0 + ncch - 1),
                            skip_group_check=True,
                        )
                # copy straight to SBUF so the PSUM bank frees early; pairs
                # of tiles share one SBUF tile so h1 can batch over both
                p, half = divmod(t, 2)
                if half == 0:
                    aggT_pairs[p] = wpool.tile([P, 2 * P], bf16, tag="aggT",
                                               name=f"aggT{p}")
                nc.scalar.activation(
                    out=aggT_pairs[p][:, half * P : (half + 1) * P],
                    in_=agg_ps[:], func=AF.Copy, bias=0.0, scale=1.0,
                )

            group_res = {}

            def group_tiles(gi):
                if gi not in group_res:
                    xb_g = xpool.tile([P, G * P], bf16, tag="xb")
                    nc.scalar.dma_start(out=xb_g[:], in_=xbf_d[gi])
                    xf_g = xpool.tile([P, G * D], f32, tag="xf")
                    nc.scalar.dma_start(out=xf_g[:], in_=xf_d[gi])
                    y_g = ypool.tile([P, G * D], f32)
                    group_res[gi] = (xb_g, xf_g, y_g)
                return group_res[gi]

            def mlp_h1_pair(p):
                """h1 for tiles (2p, 2p+1) batched over the node axis."""
                t0 = 2 * p
                nt = min(2, NT - t0)
                gi0, ti0 = divmod(t0, G)
                xb_g, _, _ = group_tiles(gi0)
                aggT = aggT_pairs.pop(p)
                NN = nt * P
                if ti0 + nt <= G:
                    xT = xb_g[:, ti0 * P : (ti0 + nt) * P]
                else:
                    # pair straddles a group boundary: stitch a pair tile
                    xT2 = wpool.tile([P, 2 * P], bf16, tag="xT2")
                    nc.vector.tensor_copy(out=xT2[:, 0:P],
                                          in_=xb_g[:, (G - 1) * P : G * P])
                    xb_g1, _, _ = group_tiles(gi0 + 1)
                    nc.vector.tensor_copy(out=xT2[:, P : 2 * P],
                                          in_=xb_g1[:, 0:P])
                    xT = xT2[:, 0:NN]

                h1a_ps = pspool.tile([P, 2 * P], f32, tag="h1a")
                nc.tensor.matmul(h1a_ps[:, 0:NN], lhsT=w1xa[:], rhs=xT,
                                 start=True, stop=False)
                nc.tensor.matmul(h1a_ps[:, 0:NN], lhsT=w1ga[:],
                                 rhs=aggT[:, 0:NN], start=False, stop=True)
                h1a = wpool.tile([P, 2 * P], bf16, tag="h1a_sb")
                nc.scalar.activation(out=h1a[:, 0:NN], in_=h1a_ps[:, 0:NN],
                                     func=AF.Relu, bias=b1a[:], scale=1.0)

                h1b_ps = pspool.tile([P, 2 * P], f32, tag="h1b")
                nc.tensor.matmul(h1b_ps[:, 0:NN], lhsT=w1xb[:], rhs=xT,
                                 start=True, stop=False)
                nc.tensor.matmul(h1b_ps[:, 0:NN], lhsT=w1gb[:],
                                 rhs=aggT[:, 0:NN], start=False, stop=True)
                h1b = wpool.tile([P, 2 * P], bf16, tag="h1b_sb")
                nc.scalar.activation(out=h1b[:, 0:NN], in_=h1b_ps[:, 0:NN],
                                     func=AF.Relu, bias=b1b[:], scale=1.0)
                return h1a, h1b

            def mlp_h2ln(t, h1a, h1b, half):
                gi, ti = divmod(t, G)
                xb_g, xf_g, y_g = group_tiles(gi)

                # h2 in node-major: h2[n, o] = sum_h h1_T[h, n] * W2[h, o]
                h2_ps = ps2pool.tile([P, P], f32, tag="h2")
                nc.tensor.matmul(h2_ps[:],
                                 lhsT=h1a[:, half * P : (half + 1) * P],
                                 rhs=w2a[:], start=True, stop=False)
                nc.tensor.matmul(h2_ps[:],
                                 lhsT=h1b[:, half * P : (half + 1) * P],
                                 rhs=w2b[:], start=False, stop=False)
                nc.tensor.matmul(h2_ps[:], lhsT=ones_row[:], rhs=b2r_sb[:],
                                 start=False, stop=True)

                # ---- LayerNorm (node-major) ----
                v_sb = wpool.tile([P, P], bf16, tag="v_sb")
                nc.vector.tensor_copy(out=v_sb[:], in_=h2_ps[:])
                stats = wpool.tile([P, 6], f32, tag="stats")
                nc.vector.bn_stats(out=stats[:], in_=v_sb[:])
                mv = wpool.tile([P, 2], f32, tag="mv")
                nc.vector.bn_aggr(out=mv[:], in_=stats[:])
                sd = wpool.tile([P, 1], f32, tag="sd")
                nc.scalar.activation(out=sd[:], in_=mv[:, 1:2],
                                     func=AF.Sqrt, bias=eps_sb[:], scale=1.0)
                rstd = wpool.tile([P, 1], f32, tag="rstd")
                nc.vector.reciprocal(out=rstd[:], in_=sd[:])
                t1 = wpool.tile([P, P], bf16, tag="t1")
                nc.vector.tensor_scalar(
                    out=t1[:], in0=v_sb[:], scalar1=mv[:, 0:1],
                    scalar2=rstd[:], op0=OP.subtract, op1=OP.mult,
                )
                t2 = wpool.tile([P, P], f32, tag="t2")
                t2_eng.tensor_tensor(out=t2[:], in0=t1[:], in1=gb_sb[:],
                                     op=OP.mult)
                y_eng.tensor_tensor(
                    out=y_g[:, ti * D : (ti + 1) * D],
                    in0=t2[:],
                    in1=xf_g[:, ti * D : (ti + 1) * D],
                    op=OP.add,
                )
                if ti == G - 1:
                    nc.scalar.dma_start(out=out_d[gi], in_=y_g[:])
                    del group_res[gi]

            # software pipeline: S-builds run SA tiles ahead of the scatter
            # matmuls, which run MA tiles ahead of the MLP/LayerNorm
            SA, MA = 8, 4
            for t in range(min(SA, NT)):
                sbuild_tile(t)
            for t in range(min(MA, NT)):
                scatter_tile(t)
            npairs = (NT + 1) // 2
            for p in range(npairs):
                t0 = 2 * p
                for t in (t0, t0 + 1):
                    if t + SA < NT:
                        sbuild_tile(t + SA)
                # h1 first so its relus enter the ACT queue ahead of the
                # next tiles' aggT copies; the scatter matmuls then cover
                # the relu latency before h2 needs h1 as weights
                h1a, h1b = mlp_h1_pair(p)
                for t in (t0, t0 + 1):
                    if t + MA < NT:
                        scatter_tile(t + MA)
                mlp_h2ln(t0, h1a, h1b, 0)
                if t0 + 1 < NT:
                    mlp_h2ln(t0 + 1, h1a, h1b, 1)

    nc.finalize()
    return nc


LAST_RESULT = None


def kernel(x, edge_index, edge_attr, W1, b1, W2, b2, ln_g, ln_b):
    global LAST_RESULT
    in_maps, meta, tile_perms = _prep_host(
        x, edge_index, edge_attr, W1, b1, W2, b2, ln_g, ln_b
    )
    nc = _build_program(meta)
    trace = bool(os.environ.get("KERNEL_TRACE"))
    res = run_bass_kernel_spmd(
        nc, in_maps, core_ids=list(range(NCORE)), trace=trace
    )
    LAST_RESULT = res

    out = np.empty((N_NODES, D), dtype=np.float32)
    for c in range(NCORE):
        yN = res.results[c]["outN"]  # [G, P, G*D] node-major, slot order
        y_slots = yN.reshape(G, P, G, D).transpose(0, 2, 1, 3).reshape(NT, P, D)
        y_tiles = np.empty_like(y_slots)
        y_tiles[tile_perms[c]] = y_slots
        y = y_tiles.reshape(NPAD, D)[:NSHARD]
        out[c * NSHARD : (c + 1) * NSHARD] = y
    return out



# revision 8
# speedup vs baseline: 1.0873x; 1.0873x over previous
"""Trainium2 Bass kernel for nn_NodeProcessor (GNN message passing).

Strategy (8 NeuronCores, SPMD, no collectives):
  - Host sorts edges by destination node and shards NODES (6250/core);
    each core receives exactly the edges destined to its node shard, so no
    cross-core reduction is needed.
  - On device, segment-sum is computed per 128-node tile as a sequence of
    128-edge-chunk matmuls accumulating in PSUM:
        agg_T[f, n] += sum_e edge_chunk[e, f] * S[e, n],
    where S[e, n] = (j_rel[e] == n) is a one-hot selection matrix built by
    an is_equal compare against a constant iota row.  Edges are pre-sorted,
    so each chunk belongs to one node tile and spans few nodes: chunk 0 of
    a tile writes the full 128-wide region (start=True clears the
    accumulator), later chunks compare/accumulate only a W=32-wide window
    at a host-baked column offset.  S matrices are built in batched
    compare ops (16 windows / 7 full tiles at a time).
  - MLP: h1_T = relu(W1.T @ [x_T; agg_T] + b1) feature-major; h2 is then
    produced directly in node-major form by using h1_T as the stationary
    operand (h2[n, o] = sum_h h1_T[h, n] * W2[h, o]), with b2 added via a
    rank-1 ones-column matmul — no on-device transposes anywhere.
  - LayerNorm node-major via bn_stats/bn_aggr + fused (v-mu)*rstd
    tensor_scalar; gamma scale + residual add run on GpSimd; the residual
    (x + beta) is folded on host.  Output is stored node-major.
  - Per-core tile processing order is chosen (descending chunk count) so
    one SPMD program (a common per-tile chunk schedule) fits all cores.
    Scatter for tile t+1 is emitted ahead of the MLP of tile t to keep the
    in-order PE queue fed.

Matmul inputs are bf16; accumulation is f32 in PSUM; LayerNorm statistics
and the residual add are f32.
"""

import os
import sys

import numpy as np

for _p in ("/opt/trn_rl_repo", "/root/.axon_site/_ro/trn_rl_repo"):
    if os.path.isdir(_p) and _p not in sys.path:
        sys.path.insert(0, _p)

import ml_dtypes

import concourse.bacc as bacc
import concourse.bass as bass
import concourse.tile as tile
from concourse import mybir
from concourse.bass_utils import run_bass_kernel_spmd

BF16 = ml_dtypes.bfloat16
E3 = ml_dtypes.float8_e3m4

N_NODES = 50000
N_EDGES = 600000
D = 128           # node/edge feature dim
H = 256           # hidden dim
NCORE = 8
NSHARD = N_NODES // NCORE      # 6250 real nodes per core
P = 128                        # partition / tile size
NT = 49                        # node tiles per core (49*128 = 6272 >= 6250)
G = 7                          # tile group size (NT = G*G)
NPAD = NT * P                  # padded nodes per core
L = 32                         # edge chunks per DMA load
W = 32                         # scatter window width (max common span is 27)
SB = 16                        # windows per batched S-build op
LN_EPS = 1e-5
PAD_J = 200.0                  # j_rel sentinel for padded edge rows

S_ENGINE = os.environ.get("S_ENGINE", "vector")      # S-build engine
T2_ENGINE = os.environ.get("T2_ENGINE", "gpsimd")    # gamma-mult engine
Y_ENGINE = os.environ.get("Y_ENGINE", "gpsimd")      # residual-add engine

if os.environ.get("KERNEL_LDW_OPT"):
    from concourse import bass_utils as _bu

    _orig_run_command = _bu.run_command

    def _patched_run_command(argv, **kw):
        argv = [
            "--enable-ldw-opt=true" if a == "--enable-ldw-opt=false" else a
            for a in argv
        ]
        return _orig_run_command(argv, **kw)

    _bu.run_command = _patched_run_command


def _prep_host(x, edge_index, edge_attr, W1, b1, W2, b2, ln_g, ln_b):
    """Sort/shard/pack all inputs."""
    j = np.asarray(edge_index[1], dtype=np.int64)
    perm = np.argsort(j, kind="stable")
    js = j[perm]

    edge_attr_bf = np.asarray(edge_attr, dtype=E3)
    x = np.asarray(x, dtype=np.float32)
    ln_b = np.asarray(ln_b, dtype=np.float32)

    bounds = np.searchsorted(js, np.arange(NCORE + 1) * NSHARD)

    core_info = []
    for c in range(NCORE):
        es, ee = bounds[c], bounds[c + 1]
        jl = js[es:ee] - c * NSHARD           # local node id, 0..6249
        rows = perm[es:ee]                    # rows into edge_attr
        cnt = np.bincount(jl // P, minlength=NT)  # edges per tile
        ch = -(-cnt // P)                     # ceil chunks per tile
        tile_perm = np.argsort(-ch, kind="stable")  # descending chunk count
        core_info.append((jl, rows, cnt, ch, tile_perm))

    sorted_ch = np.stack([ci[3][ci[4]] for ci in core_info])  # [NCORE, NT]
    schedule = np.maximum(sorted_ch.max(axis=0), 1).astype(np.int64)
    nchunk = int(schedule.sum())
    nload = -(-nchunk // L)
    nc_tot = nload * L

    chunk_base = np.zeros(NT + 1, dtype=np.int64)
    np.cumsum(schedule, out=chunk_base[1:])

    # Tile-relative j_rel per chunk slot per core; chunk 0 of a tile is
    # full-width, later chunks use a common W-wide window (max over cores
    # of the span must fit — verified below).
    minj = np.full((NCORE, nc_tot), 1 << 30, dtype=np.int64)
    maxj = np.full((NCORE, nc_tot), -1, dtype=np.int64)
    per_core_fill = []
    for c in range(NCORE):
        jl, rows, cnt, ch, tile_perm = core_info[c]
        tile_start = np.zeros(NT + 1, dtype=np.int64)
        np.cumsum(cnt, out=tile_start[1:])
        ridx = np.zeros(nc_tot * P, dtype=np.int64)
        jrel_t = np.full(nc_tot * P, -1, dtype=np.int64)  # tile-relative
        for s in range(NT):
            T = int(tile_perm[s])
            n = int(cnt[T])
            dst = chunk_base[s] * P
            ridx[dst : dst + n] = rows[tile_start[T] : tile_start[T] + n]
            jrel_t[dst : dst + n] = jl[tile_start[T] : tile_start[T] + n] - T * P
        jr2 = jrel_t.reshape(nc_tot, P)
        valid = jr2 >= 0
        anyv = valid.any(axis=1)
        mn = np.where(anyv, np.where(valid, jr2, 1 << 30).min(axis=1), 1 << 30)
        mx = np.where(anyv, np.where(valid, jr2, -1).max(axis=1), -1)
        minj[c] = mn
        maxj[c] = mx
        per_core_fill.append((ridx, jrel_t))

    woff = np.clip(minj.min(axis=0), 0, P - W)
    woff[chunk_base[:-1]] = 0  # chunk 0 full width
    fw = np.zeros(nc_tot, dtype=bool)
    fw[chunk_base[:-1]] = True
    width = np.where(fw, P, W)
    assert (maxj.max(axis=0) < woff + width).all(), "chunk span exceeds window"

    in_maps = []
    for c in range(NCORE):
        jl, rows, cnt, ch, tile_perm = core_info[c]
        ridx, jrel_t = per_core_fill[c]
        jr2 = jrel_t.reshape(nc_tot, P).astype(np.float32) - woff[:, None]
        jr2[jrel_t.reshape(nc_tot, P) < 0] = PAD_J

        ea_all = edge_attr_bf[ridx]           # [nc_tot*P, D]
        ea_pack = (
            ea_all.reshape(nload, L, P, D)
            .transpose(0, 2, 1, 3)
            .reshape(nload, P, L * D)
            .copy()
        )
        jr_pack = np.ascontiguousarray(jr2.T.astype(BF16))  # [P, nc_tot]
        # chunk-0 columns (tile-relative j_rel) gathered into slot order
        jr0_pack = np.ascontiguousarray(jr2[chunk_base[:-1]].T.astype(BF16))

        # x shard: bf16 feature-major tiles (MLP input) and f32 node-major
        # residual (+ beta folded), ordered by tile_perm, G tiles per DMA.
        xs = np.zeros((NPAD, D), dtype=np.float32)
        xs[:NSHARD] = x[c * NSHARD : (c + 1) * NSHARD]
        xt = xs.reshape(NT, P, D).transpose(0, 2, 1)[tile_perm]  # [NT, f, n]
        xbf_pack = (
            xt.astype(BF16).reshape(G, G, D, P).transpose(0, 2, 1, 3)
            .reshape(G, D, G * P).copy()
        )
        xfn = (xs + ln_b[None, :]).reshape(NT, P, D)[tile_perm]  # [NT, n, f]
        xf_pack = (
            xfn.reshape(G, G, P, D).transpose(0, 2, 1, 3)
            .reshape(G, P, G * D).copy()
        )

        vecs = np.concatenate(
            [np.asarray(b1, np.float32).ravel()]
        ).reshape(H, 1)

        in_maps.append(
            {
                "ea": ea_pack,
                "jr": jr_pack,
                "jr0": jr0_pack,
                "xbf": xbf_pack,
                "xf": xf_pack,
                "W1d": np.asarray(W1, BF16),
                "W2d": np.asarray(W2, BF16),
                "vecs": vecs,
                "b2r": np.asarray(b2, BF16).reshape(1, D),
                "gb": np.tile(np.asarray(ln_g, np.float32), (P, 1)).astype(BF16),
                "iota_d": np.tile(np.arange(P, dtype=np.float32), (P, 1)).astype(BF16),
                "iotaw_d": np.tile(np.arange(W, dtype=np.float32), (P, SB)).astype(BF16),
                "iotag_d": np.tile(np.arange(P, dtype=np.float32), (P, G)).astype(BF16),
            }
        )

    meta = (schedule, woff, nload, nc_tot)
    return in_maps, meta, [ci[4] for ci in core_info]


def _build_program(meta):
    schedule, woff, nload, nc_tot = meta
    f32 = mybir.dt.float32
    bf16 = mybir.dt.bfloat16
    fp8 = mybir.dt.float8e3
    AF = mybir.ActivationFunctionType
    OP = mybir.AluOpType

    nc = bacc.Bacc("TRN2", target_bir_lowering=False, debug=False,
                   num_devices=NCORE)
    s_eng = getattr(nc, S_ENGINE)
    t2_eng = getattr(nc, T2_ENGINE)
    y_eng = getattr(nc, Y_ENGINE)

    ea_d = nc.dram_tensor("ea", [nload, P, L * D], fp8, kind="ExternalInput").ap()
    jr_d = nc.dram_tensor("jr", [P, nc_tot], bf16, kind="ExternalInput").ap()
    jr0_d = nc.dram_tensor("jr0", [P, NT], bf16, kind="ExternalInput").ap()
    xbf_d = nc.dram_tensor("xbf", [G, D, G * P], bf16, kind="ExternalInput").ap()
    xf_d = nc.dram_tensor("xf", [G, P, G * D], f32, kind="ExternalInput").ap()
    w1_d = nc.dram_tensor("W1d", [H, H], bf16, kind="ExternalInput").ap()
    w2_d = nc.dram_tensor("W2d", [H, D], bf16, kind="ExternalInput").ap()
    vecs_d = nc.dram_tensor("vecs", [H, 1], f32, kind="ExternalInput").ap()
    b2r_d = nc.dram_tensor("b2r", [1, D], bf16, kind="ExternalInput").ap()
    gb_d = nc.dram_tensor("gb", [P, D], bf16, kind="ExternalInput").ap()
    iota_dr = nc.dram_tensor("iota_d", [P, P], bf16, kind="ExternalInput").ap()
    iotaw_dr = nc.dram_tensor("iotaw_d", [P, SB * W], bf16, kind="ExternalInput").ap()
    iotag_dr = nc.dram_tensor("iotag_d", [P, G * P], bf16, kind="ExternalInput").ap()
    out_d = nc.dram_tensor("outN", [G, P, G * D], f32, kind="ExternalOutput").ap()

    with tile.TileContext(nc) as tc:
        with (
            tc.tile_pool(name="consts", bufs=1) as consts,
            tc.tile_pool(name="edges", bufs=6) as epool,
            tc.tile_pool(name="xg", bufs=2) as xpool,
            tc.tile_pool(name="yg", bufs=2) as ypool,
            tc.tile_pool(name="s0", bufs=3) as s0pool,
            tc.tile_pool(name="sm", bufs=8) as spool,
            tc.tile_pool(name="work", bufs=3) as wpool,
            tc.tile_pool(name="ps", bufs=1, space="PSUM") as pspool,
            tc.tile_pool(name="ps2", bufs=3, space="PSUM") as ps2pool,
            tc.tile_pool(name="psagg", bufs=3, space="PSUM") as psagg,
        ):
            # ---- constants ----
            jr_sb = consts.tile([P, nc_tot], bf16)
            nc.sync.dma_start(out=jr_sb[:], in_=jr_d[:])
            jr0_sb = consts.tile([P, NT], bf16, tag="jr0")
            nc.sync.dma_start(out=jr0_sb[:], in_=jr0_d[:])
            iota_sb = consts.tile([P, P], bf16)
            nc.sync.dma_start(out=iota_sb[:], in_=iota_dr[:])
            iotaw_sb = consts.tile([P, SB * W], bf16, tag="iotaw")
            nc.sync.dma_start(out=iotaw_sb[:], in_=iotaw_dr[:])
            iotag_sb = consts.tile([P, G * P], bf16, tag="iotag")
            nc.sync.dma_start(out=iotag_sb[:], in_=iotag_dr[:])
            gb_sb = consts.tile([P, D], bf16, tag="gb")
            nc.sync.dma_start(out=gb_sb[:], in_=gb_d[:])
            b2r_sb = consts.tile([1, D], bf16, tag="b2r")
            nc.sync.dma_start(out=b2r_sb[:], in_=b2r_d[:])
            ones_row = consts.tile([1, P], bf16, tag="ones_row")
            nc.vector.memset(ones_row[:], 1.0)

            w1xa = consts.tile([P, P], bf16, tag="w1xa")
            nc.sync.dma_start(out=w1xa[:], in_=w1_d[0:P, 0:P])
            w1xb = consts.tile([P, P], bf16, tag="w1xb")
            nc.sync.dma_start(out=w1xb[:], in_=w1_d[0:P, P : 2 * P])
            w1ga = consts.tile([P, P], bf16, tag="w1ga")
            nc.sync.dma_start(out=w1ga[:], in_=w1_d[P : 2 * P, 0:P])
            w1gb = consts.tile([P, P], bf16, tag="w1gb")
            nc.sync.dma_start(out=w1gb[:], in_=w1_d[P : 2 * P, P : 2 * P])
            w2a = consts.tile([P, P], bf16, tag="w2a")
            nc.sync.dma_start(out=w2a[:], in_=w2_d[0:P, :])
            w2b = consts.tile([P, P], bf16, tag="w2b")
            nc.sync.dma_start(out=w2b[:], in_=w2_d[P : 2 * P, :])

            b1a = consts.tile([P, 1], f32, tag="b1a")
            nc.sync.dma_start(out=b1a[:], in_=vecs_d[0:P, :])
            b1b = consts.tile([P, 1], f32, tag="b1b")
            nc.sync.dma_start(out=b1b[:], in_=vecs_d[P : 2 * P, :])
            eps_sb = consts.tile([P, 1], f32, tag="eps")
            nc.vector.memset(eps_sb[:], LN_EPS)

            def iota3(qn, width):
                a = iota_sb[:, 0:width]
                return bass.AP(tensor=a.tensor, offset=a.offset,
                               ap=[a.ap[0], [0, qn], a.ap[1]])

            load_tiles = {}

            def ensure_load(ld):
                if ld < 0 or ld >= nload or ld in load_tiles:
                    return
                et = epool.tile([P, L * D], fp8, tag="ea", name=f"ea{ld}")
                nc.sync.dma_start(out=et[:], in_=ea_d[ld])
                load_tiles[ld] = et

            def edge_slice(c):
                ld, sl = divmod(c, L)
                ensure_load(ld)
                ensure_load(ld + 1)
                ensure_load(ld + 2)
                return load_tiles[ld][:, sl * D : (sl + 1) * D]

            chunk_base = np.zeros(NT + 1, dtype=np.int64)
            np.cumsum(schedule, out=chunk_base[1:])

            # batched full-width S for the chunk-0s of one tile group
            s0_tiles = {}

            def s0_group(gi):
                if gi not in s0_tiles:
                    S0g = s0pool.tile([P, G * P], fp8, tag="S0g")
                    s_eng.tensor_tensor(
                        out=S0g[:].rearrange("p (q w) -> p q w", w=P),
                        in0=jr0_sb[:, gi * G : (gi + 1) * G].to_broadcast(
                            [P, G, P]
                        ),
                        in1=iotag_sb[:].rearrange("p (q w) -> p q w", w=P),
                        op=OP.is_equal,
                    )
                    s0_tiles[gi] = S0g
                return s0_tiles[gi]

            aggT_pairs = {}
            s_of = {}

            def sbuild_tile(t):
                """Build the selection matrices for tile t (runs well ahead
                of the matmuls so the in-order DVE queue never gates PE)."""
                gi, ti = divmod(t, G)
                c0 = int(chunk_base[t])
                ncch = int(schedule[t])
                s0_group(gi)
                sbs = []
                for q0 in range(1, ncch, SB):
                    qn = min(SB, ncch - q0)
                    Sb = spool.tile([P, SB * W], fp8, tag="Sb",
                                    name=f"Sb{t}_{q0}")
                    s_eng.tensor_tensor(
                        out=Sb[:, : qn * W].rearrange("p (q w) -> p q w", w=W),
                        in0=jr_sb[:, c0 + q0 : c0 + q0 + qn].to_broadcast(
                            [P, qn, W]
                        ),
                        in1=iotaw_sb[:, : qn * W].rearrange(
                            "p (q w) -> p q w", w=W
                        ),
                        op=OP.is_equal,
                    )
                    sbs.append(Sb)
                s_of[t] = sbs

            def scatter_tile(t):
                gi, ti = divmod(t, G)
                c0 = int(chunk_base[t])
                ncch = int(schedule[t])
                agg_ps = psagg.tile([P, P], f32, tag="agg")
                S0g = s0_group(gi)
                nc.tensor.matmul(
                    agg_ps[:], lhsT=edge_slice(c0),
                    rhs=S0g[:, ti * P : (ti + 1) * P],
                    start=True, stop=(ncch == 1),
                )
                sbs = s_of.pop(t)
                for bi, q0 in enumerate(range(1, ncch, SB)):
                    qn = min(SB, ncch - q0)
                    Sb = sbs[bi]
                    for i in range(qn):
                        c = c0 + q0 + i
                        w = int(woff[c])
                        nc.tensor.matmul(
                            agg_ps[:, w : w + W],
                            lhsT=edge_slice(c),
                            rhs=Sb[:, i * W : (i + 1) * W],
                            start=False,
                            stop=(c == c0 + ncch - 1),
                            skip_group_check=True,
                        )
                # copy straight to SBUF so the PSUM bank frees early; pairs
                # of tiles share one SBUF tile so h1 can batch over both
                p, half = divmod(t, 2)
                if half == 0:
                    aggT_pairs[p] = wpool.tile([P, 2 * P], bf16, tag="aggT",
                                               name=f"aggT{p}")
                nc.scalar.activation(
                    out=aggT_pairs[p][:, half * P : (half + 1) * P],
                    in_=agg_ps[:], func=AF.Copy, bias=0.0, scale=1.0,
                )

            group_res = {}

            def group_tiles(gi):
                if gi not in group_res:
                    xb_g = xpool.tile([P, G * P], bf16, tag="xb")
                    nc.scalar.dma_start(out=xb_g[:], in_=xbf_d[gi])
                    xf_g = xpool.tile([P, G * D], f32, tag="xf")
                    nc.scalar.dma_start(out=xf_g[:], in_=xf_d[gi])
                    y_g = ypool.tile([P, G * D], f32)
                    group_res[gi] = (xb_g, xf_g, y_g)
                return group_res[gi]

            def mlp_h1_pair(p):
                """h1 for tiles (2p, 2p+1) batched over the node axis."""
                t0 = 2 * p
                nt = min(2, NT - t0)
                gi0, ti0 = divmod(t0, G)
                xb_g, _, _ = group_tiles(gi0)
                aggT = aggT_pairs.pop(p)
                NN = nt * P
                if ti0 + nt <= G:
                    xT = xb_g[:, ti0 * P : (ti0 + nt) * P]
                else:
                    # pair straddles a group boundary: stitch a pair tile
                    xT2 = wpool.tile([P, 2 * P], bf16, tag="xT2")
                    nc.vector.tensor_copy(out=xT2[:, 0:P],
                                          in_=xb_g[:, (G - 1) * P : G * P])
                    xb_g1, _, _ = group_tiles(gi0 + 1)
                    nc.vector.tensor_copy(out=xT2[:, P : 2 * P],
                                          in_=xb_g1[:, 0:P])
                    xT = xT2[:, 0:NN]

                h1a_ps = pspool.tile([P, 2 * P], f32, tag="h1a")
                nc.tensor.matmul(h1a_ps[:, 0:NN], lhsT=w1xa[:], rhs=xT,
                                 start=True, stop=False)
                nc.tensor.matmul(h1a_ps[:, 0:NN], lhsT=w1ga[:],
                                 rhs=aggT[:, 0:NN], start=False, stop=True)
                h1a = wpool.tile([P, 2 * P], bf16, tag="h1a_sb")
                nc.scalar.activation(out=h1a[:, 0:NN], in_=h1a_ps[:, 0:NN],
                                     func=AF.Relu, bias=b1a[:], scale=1.0)

                h1b_ps = pspool.tile([P, 2 * P], f32, tag="h1b")
                nc.tensor.matmul(h1b_ps[:, 0:NN], lhsT=w1xb[:], rhs=xT,
                                 start=True, stop=False)
                nc.tensor.matmul(h1b_ps[:, 0:NN], lhsT=w1gb[:],
                                 rhs=aggT[:, 0:NN], start=False, stop=True)
                h1b = wpool.tile([P, 2 * P], bf16, tag="h1b_sb")
                nc.scalar.activation(out=h1b[:, 0:NN], in_=h1b_ps[:, 0:NN],
                                     func=AF.Relu, bias=b1b[:], scale=1.0)
                return h1a, h1b

            def mlp_h2ln(t, h1a, h1b, half):
                gi, ti = divmod(t, G)
                xb_g, xf_g, y_g = group_tiles(gi)

                # h2 in node-major: h2[n, o] = sum_h h1_T[h, n] * W2[h, o]
                h2_ps = ps2pool.tile([P, P], f32, tag="h2")
                nc.tensor.matmul(h2_ps[:],
                                 lhsT=h1a[:, half * P : (half + 1) * P],
                                 rhs=w2a[:], start=True, stop=False)
                nc.tensor.matmul(h2_ps[:],
                                 lhsT=h1b[:, half * P : (half + 1) * P],
                                 rhs=w2b[:], start=False, stop=False)
                nc.tensor.matmul(h2_ps[:], lhsT=ones_row[:], rhs=b2r_sb[:],
                                 start=False, stop=True)

                # ---- LayerNorm (node-major) ----
                v_sb = wpool.tile([P, P], bf16, tag="v_sb")
                nc.vector.tensor_copy(out=v_sb[:], in_=h2_ps[:])
                stats = wpool.tile([P, 6], f32, tag="stats")
                nc.vector.bn_stats(out=stats[:], in_=v_sb[:])
                mv = wpool.tile([P, 2], f32, tag="mv")
                nc.vector.bn_aggr(out=mv[:], in_=stats[:])
                sd = wpool.tile([P, 1], f32, tag="sd")
                nc.scalar.activation(out=sd[:], in_=mv[:, 1:2],
                                     func=AF.Sqrt, bias=eps_sb[:], scale=1.0)
                rstd = wpool.tile([P, 1], f32, tag="rstd")
                nc.vector.reciprocal(out=rstd[:], in_=sd[:])
                t1 = wpool.tile([P, P], bf16, tag="t1")
                nc.vector.tensor_scalar(
                    out=t1[:], in0=v_sb[:], scalar1=mv[:, 0:1],
                    scalar2=rstd[:], op0=OP.subtract, op1=OP.mult,
                )
                t2 = wpool.tile([P, P], f32, tag="t2")
                t2_eng.tensor_tensor(out=t2[:], in0=t1[:], in1=gb_sb[:],
                                     op=OP.mult)
                y_eng.tensor_tensor(
                    out=y_g[:, ti * D : (ti + 1) * D],
                    in0=t2[:],
                    in1=xf_g[:, ti * D : (ti + 1) * D],
                    op=OP.add,
                )
                if ti == G - 1:
                    nc.scalar.dma_start(out=out_d[gi], in_=y_g[:])
                    del group_res[gi]

            # software pipeline: S-builds run SA tiles ahead of the scatter
            # matmuls, which run MA tiles ahead of the MLP/LayerNorm
            SA, MA = 8, 4
            for t in range(min(SA, NT)):
                sbuild_tile(t)
            for t in range(min(MA, NT)):
                scatter_tile(t)
            npairs = (NT + 1) // 2
            for p in range(npairs):
                t0 = 2 * p
                for t in (t0, t0 + 1):
                    if t + SA < NT:
                        sbuild_tile(t + SA)
                # h1 first so its relus enter the ACT queue ahead of the
                # next tiles' aggT copies; the scatter matmuls then cover
                # the relu latency before h2 needs h1 as weights
                h1a, h1b = mlp_h1_pair(p)
                for t in (t0, t0 + 1):
                    if t + MA < NT:
                        scatter_tile(t + MA)
                mlp_h2ln(t0, h1a, h1b, 0)
                if t0 + 1 < NT:
                    mlp_h2ln(t0 + 1, h1a, h1b, 1)

    nc.finalize()
    return nc


LAST_RESULT = None


def kernel(x, edge_index, edge_attr, W1, b1, W2, b2, ln_g, ln_b):
    global LAST_RESULT
    in_maps, meta, tile_perms = _prep_host(
        x, edge_index, edge_attr, W1, b1, W2, b2, ln_g, ln_b
    )
    nc = _build_program(meta)
    trace = bool(os.environ.get("KERNEL_TRACE"))
    res = run_bass_kernel_spmd(
        nc, in_maps, core_ids=list(range(NCORE)), trace=trace
    )
    LAST_RESULT = res

    out = np.empty((N_NODES, D), dtype=np.float32)
    for c in range(NCORE):
        yN = res.results[c]["outN"]  # [G, P, G*D] node-major, slot order
        y_slots = yN.reshape(G, P, G, D).transpose(0, 2, 1, 3).reshape(NT, P, D)
        y_tiles = np.empty_like(y_slots)
        y_tiles[tile_perms[c]] = y_slots
        y = y_tiles.reshape(NPAD, D)[:NSHARD]
        out[c * NSHARD : (c + 1) * NSHARD] = y
    return out

